# revision 30
# baseline (speedup 1.0000x reference)
"""Trainium2 Bass kernel for ChunkedMultiHeadCardPassingLayer (B=4, T=4096, C=1024).

Sharding: 8 cores = B(4) x T-halves(2). Each core computes output rows
[g*2048, (g+1)*2048) of batch b; the only cross-core dependency is the
chunk-carry running sum, exchanged with a tiny pairwise AllGather.

Structure (v2):
  - mark/gate matmul runs in fp8e4 DoubleRow perf mode at 2x PE rate:
    the stationary x is split hi/lo across the two DoubleRow planes
    (x = hi + lo/16, both fp8) so x keeps ~11 mantissa bits; the moving
    weights ride both planes as [16W | W], and the 1/16 unscale folds
    into the sigmoid scale and the gated-mark multiply.
  - the carry row is LayerNormed then DMA'd into partition 127 of the
    gm tile; the strict-cumsum stationary has row 127 = all-ones, so a
    single tri matmul per chunk produces cumsum+carry (no K=1 carry
    matmuls, no partition-spread DMAs).
  - card LN stats come from bn_stats (one DVE pass, even/odd combined
    in registers-sized ops), batched per 4-chunk group; the proj-LN
    row sums ride the activation-copy accum_out for free.
  - loop2 is software-pipelined in groups of 4 chunks: stats of group
    g interleave with the MLP of group g-1; PSUM rotates {q, ho1, pj}
    through 3 bufs plus a shared zt/hop slot (exactly 8 banks).
  - residual add + final affine run on the (otherwise idle) gpsimd.
"""

import sys

sys.path.insert(0, "/opt/trn_rl_repo")

import numpy as np
import ml_dtypes

import concourse.bass as bass
import concourse.tile as tile
from concourse import bacc, bass_isa, mybir
from concourse.bass_utils import run_bass_kernel_spmd
from concourse.dve_ops import AFFINE_THEN_ADD

F32 = mybir.dt.float32
BF16 = mybir.dt.bfloat16
I32 = mybir.dt.int32
U8 = mybir.dt.uint8
F8 = mybir.dt.float8e4
AL = mybir.AluOpType
AF = mybir.ActivationFunctionType
X = mybir.AxisListType.X
DR = mybir.MatmulPerfMode.DoubleRow
BFNP = ml_dtypes.bfloat16
F8NP = ml_dtypes.float8_e4m3

B, T, C = 4, 4096, 1024
H, CS, D = 16, 128, 64
EPS = 1e-5
NCORES = 8
TL = T // 2          # rows per core
NCH = TL // CS       # chunks per core
RSQRT_MAGIC = 0x5F3759DF
GS = 4               # chunks per pipeline group
NG = NCH // GS


def _newton_rsqrt(nc, pool, v, p, n, tag, iters=2):
    """y = 1/sqrt(v) elementwise for v > 0, [p, n] fp32, vector engine only."""
    y = pool.tile([p, n], F32, name=f"nry_{tag}")
    ti = pool.tile([p, n], I32, name=f"nri_{tag}")
    nc.vector.tensor_scalar(ti[:], v.bitcast(I32), 1, None, op0=AL.logical_shift_right)
    nc.vector.tensor_scalar(ti[:], ti[:], -1, None, op0=AL.mult)
    nc.vector.tensor_scalar(y[:].bitcast(I32), ti[:], RSQRT_MAGIC, None, op0=AL.add)
    nh = pool.tile([p, n], F32, name=f"nrh_{tag}")
    nc.vector.tensor_scalar(nh[:], v, -0.5, None, op0=AL.mult)
    ysq = pool.tile([p, n], F32, name=f"nrq_{tag}")
    for _ in range(iters):
        # y <- y * (1.5 + (-0.5 v) * y^2)
        nc.vector.tensor_tensor(ysq[:], y[:], y[:], op=AL.mult)
        nc.vector.tensor_tensor(ysq[:], ysq[:], nh[:], op=AL.mult)
        nc.vector.scalar_tensor_tensor(y[:], ysq[:], 1.5, y[:],
                                       op0=AL.add, op1=AL.mult)
    return y


def _dr_view(t, col_off, ncols):
    """3-D DoubleRow AP over a [128, 2*P] plane-major u8 tile: planes at
    free offsets {0, P}, slice [col_off : col_off+ncols] of each plane."""
    a = t[:]
    plane = a.ap[-1][1] // 2
    ap = bass.AP(a.tensor, a.offset + col_off,
                 [a.ap[0], [plane, 2], [1, ncols]])
    return ap.bitcast(F8)


def build_nc(flags):
    """flags: (mgb, projb, h1b, h2b, lng, carry_gb) nonzero-emission booleans."""
    f_mgb, f_projb, f_h1b, f_h2b, f_lng, f_cgb = flags
    nc = bacc.Bacc("TRN2", target_bir_lowering=False, debug=False, num_devices=NCORES)

    dram_in = lambda n, s, d: nc.dram_tensor(n, s, d, kind="ExternalInput").ap()
    # c-block-major packing: one DMA per tensor instead of 8 (SP dispatch is
    # ~1.6us per dma_start and throttles the loop1 ramp otherwise)
    xhl = dram_in("xhl", [128, 8 * 2 * TL], U8)    # fp8 x hi | lo*16, [c, t]
    xT = dram_in("xT", [128, 8 * TL], BF16)        # bf16 x for the head MLP
    xres = dram_in("xres", [TL, C], BF16)
    wmg8 = dram_in("wmg8", [128, 8 * 4 * C], U8)   # fp8 [16*Wmg | Wmg] planes
    wproj = dram_in("wproj", [128, 8 * C], BF16)
    w1x = dram_in("w1x", [2 * D, 2 * D], BF16)
    w1z = dram_in("w1z", [2 * D, 2 * D], BF16)
    w2 = dram_in("w2", [2 * D, D], BF16)
    tri = dram_in("tri", [CS, CS], BF16)       # strict-upper + carry row 127
    tri16 = dram_in("tri16", [NCH, NCH], BF16)
    ones16 = dram_in("ones16", [NCH, 1], BF16)
    ones1 = dram_in("ones1", [CS, CS], BF16)
    ejs = dram_in("ejs", [CS, NCH * NCH], BF16)
    ident = dram_in("ident", [CS, CS], BF16)
    tsel = dram_in("tsel", [1, 1], F32)
    if f_mgb:
        wmgb = dram_in("wmgb", [1, 2 * C], BF16)   # pre-scaled by 16
    if f_projb:
        wprojb = dram_in("wprojb", [1, C], BF16)
    if f_h1b:
        w1b = dram_in("w1b", [1, 2 * D], BF16)
    if f_h2b:
        w2b = dram_in("w2b", [1, D], BF16)
    if f_h1b or f_h2b:
        onesN = dram_in("onesN", [1, 8 * CS], BF16)
    if f_lng:
        lngb = dram_in("lngb", [128, C], F32)
    if f_cgb:
        cgb = dram_in("cgb", [NCH, 2 * C], F32)
    out = nc.dram_tensor("out", [TL, C], BF16, kind="ExternalOutput").ap()

    with tile.TileContext(nc) as tc:
        with tc.tile_pool(name="const", bufs=1) as cp, \
             tc.tile_pool(name="bigbf", bufs=20) as bb, \
             tc.tile_pool(name="stats", bufs=1) as stp, \
             tc.tile_pool(name="stream", bufs=2) as strm, \
             tc.tile_pool(name="dram", bufs=1, space="DRAM") as dram:

            # ---------- resident weights & constants ----------
            xhl_t, wmg_t, wproj_t = [], [], []
            for c in range(8):
                xhl_t.append(cp.tile([128, 2 * TL], U8, name=f"xhl_{c}",
                                     tag=f"xsl{c}"))
                wmg_t.append(cp.tile([128, 4 * C], U8, name=f"wmg8_{c}",
                                     tag=f"wsl{c}"))
                wproj_t.append(cp.tile([128, C], BF16, name=f"wprojt_{c}"))
            for c in range(8):
                nc.sync.dma_start(xhl_t[c][:],
                                  xhl[:, c * 2 * TL:(c + 1) * 2 * TL])
                nc.sync.dma_start(wmg_t[c][:],
                                  wmg8[:, c * 4 * C:(c + 1) * 4 * C])
                nc.sync.dma_start(wproj_t[c][:],
                                  wproj[:, c * C:(c + 1) * C])

            def load_const(name, src, shape):
                t = cp.tile(shape, BF16, name=name)
                nc.sync.dma_start(t[:], src[:])
                return t

            w1x_t = load_const("w1xt", w1x, [2 * D, 2 * D])
            w1z_t = load_const("w1zt", w1z, [2 * D, 2 * D])
            w2_t = load_const("w2t", w2, [2 * D, D])
            tri_t = load_const("trit", tri, [CS, CS])
            tri16_t = load_const("tri16t", tri16, [NCH, NCH])
            ones16_t = load_const("ones16t", ones16, [NCH, 1])
            ones1_t = load_const("ones1t", ones1, [CS, CS])
            ejs_t = load_const("ejst", ejs, [CS, NCH * NCH])
            ident_t = load_const("identt", ident, [CS, CS])
            tsel_t = cp.tile([1, 1], F32, name="tselt")
            nc.sync.dma_start(tsel_t[:], tsel[:])
            if f_mgb:
                wmgb_t = load_const("wmgbt", wmgb, [1, 2 * C])
            if f_projb:
                wprojb_t = load_const("wprojbt", wprojb, [1, C])
            if f_h1b:
                w1b_t = load_const("w1bt", w1b, [1, 2 * D])
            if f_h2b:
                w2b_t = load_const("w2bt", w2b, [1, D])
            if f_h1b or f_h2b:
                onesN_t = load_const("onesNt", onesN, [1, 8 * CS])
            if f_lng:
                lngb_t = cp.tile([128, C], F32, name="lngbt")
                nc.sync.dma_start(lngb_t[:], lngb[:])
            if f_cgb:
                cgb_t = cp.tile([NCH, 2 * C], F32, name="cgbt")
                nc.sync.dma_start(cgb_t[:], cgb[:])

            # all chunks' gated marks in one tile so the carry row lands with
            # a single DMA (chunk j at cols [j*C, (j+1)*C), carry on part 127)
            gmbig = cp.tile([128, NCH * C], BF16, name="gmbig")

            # ============ loop1: mark/gate (fp8 DoubleRow) -> gm -> totals ===
            csum_sb = stp.tile([NCH, C], BF16, name="csum_sb")
            with tc.tile_pool(name="ps1", bufs=4, space="PSUM") as ps1:
                for j in range(NCH):
                    pm = ps1.tile([128, C], F32, name="pm", tag="ps1t")
                    gt = ps1.tile([128, C], F32, name="gt", tag="ps1t")
                    for c in range(8):
                        st = _dr_view(xhl_t[c], j * CS, CS)
                        last = (c == 7) and not f_mgb
                        nc.tensor.matmul(pm[:, 0:512], st, _dr_view(wmg_t[c], 0, 512),
                                         start=(c == 0), stop=last, perf_mode=DR)
                        nc.tensor.matmul(pm[:, 512:1024], st, _dr_view(wmg_t[c], 512, 512),
                                         start=(c == 0), stop=last, perf_mode=DR)
                        nc.tensor.matmul(gt[:, 0:512], st, _dr_view(wmg_t[c], 1024, 512),
                                         start=(c == 0), stop=last, perf_mode=DR)
                        nc.tensor.matmul(gt[:, 512:1024], st, _dr_view(wmg_t[c], 1536, 512),
                                         start=(c == 0), stop=last, perf_mode=DR)
                    if f_mgb:
                        nc.tensor.matmul(pm[:, 0:512], ones1_t[0:1, :], wmgb_t[0:1, 0:512],
                                         start=False, stop=True)
                        nc.tensor.matmul(pm[:, 512:1024], ones1_t[0:1, :],
                                         wmgb_t[0:1, 512:1024], start=False, stop=True)
                        nc.tensor.matmul(gt[:, 0:512], ones1_t[0:1, :],
                                         wmgb_t[0:1, 1024:1536], start=False, stop=True)
                        nc.tensor.matmul(gt[:, 512:1024], ones1_t[0:1, :],
                                         wmgb_t[0:1, 1536:2048], start=False, stop=True)
                    gts = bb.tile([128, C], BF16, name="gts", tag="gts", bufs=2)
                    nc.scalar.activation(gts[:], gt[:], AF.Sigmoid, scale=1.0 / 16)
                    gm = gmbig[:, j * C:(j + 1) * C]
                    # gm = (pm/16) * sigmoid(gt/16)
                    nc.vector.scalar_tensor_tensor(gm, pm[:], 1.0 / 16, gts[:],
                                                   op0=AL.mult, op1=AL.mult)
                    # chunk total on the (idle) gpsimd: no PSUM, no ej matmul.
                    # partition_all_reduce is the HW-tuned partition reduction;
                    # a tiny DMA parks row 0 in csum row j for the carries
                    # matmul.
                    ctmp = bb.tile([128, C], BF16, name="ctmp", tag="ctmp", bufs=2)
                    nc.gpsimd.partition_all_reduce(ctmp[:], gm, channels=128,
                                                   reduce_op=bass_isa.ReduceOp.add)
                    nc.gpsimd.dma_start(csum_sb[j:j + 1, :], ctmp[0:1, :])

            # ============ mid: pairwise AllGather + carry LayerNorm ==========
            with tc.tile_pool(name="psm", bufs=1, space="PSUM") as psm:
                tot = psm.tile([1, C], F32, name="tot")
                nc.tensor.matmul(tot[:, 0:512], ones16_t[:], csum_sb[:, 0:512],
                                 start=True, stop=True)
                nc.tensor.matmul(tot[:, 512:1024], ones16_t[:], csum_sb[:, 512:1024],
                                 start=True, stop=True)
                tot_sb = stp.tile([1, C], F32, name="tot_sb")
                nc.vector.tensor_copy(tot_sb[:], tot[:])

                cc_in = dram.tile([1, C], F32, name="cc_in")
                cc_out = dram.tile([2, C], F32, name="cc_out")
                nc.sync.dma_start(cc_in[:], tot_sb[:])
                nc.gpsimd.collective_compute(
                    "AllGather", AL.bypass,
                    replica_groups=[[0, 1], [2, 3], [4, 5], [6, 7]],
                    ins=[cc_in.opt()], outs=[cc_out.opt()],
                )
                # bf16 x for the head MLP reuses the fp8 x slots (dead after
                # loop1); issued behind the collective input so its 4MB does
                # not delay the exchange, and on the Pool DGE queue
                xb_t = []
                for c in range(8):
                    t1 = cp.tile([128, TL], BF16, name=f"xb_{c}", tag=f"xsl{c}")
                    nc.scalar.dma_start(t1[:], xT[:, c * TL:(c + 1) * TL])
                    xb_t.append(t1)
                gath = stp.tile([2, C], F32, name="gath")
                nc.sync.dma_start(gath[:], cc_out[:])
                carry_in = stp.tile([1, C], BF16, name="carry_in")
                nc.vector.tensor_scalar(carry_in[:], gath[0:1, :], tsel_t[0:1, 0:1],
                                        None, op0=AL.mult)

                carries = psm.tile([NCH, C], F32, name="carries")
                nc.tensor.matmul(carries[:, 0:512], tri16_t[:], csum_sb[:, 0:512],
                                 start=True, stop=False)
                nc.tensor.matmul(carries[:, 512:1024], tri16_t[:],
                                 csum_sb[:, 512:1024], start=True, stop=False)
                nc.tensor.matmul(carries[:, 0:512], ones1_t[0:1, 0:NCH],
                                 carry_in[0:1, 0:512], start=False, stop=True)
                nc.tensor.matmul(carries[:, 512:1024], ones1_t[0:1, 0:NCH],
                                 carry_in[0:1, 512:1024], start=False, stop=True)

                cS1 = stp.tile([NCH, H], F32, name="cS1")
                nc.vector.reduce_sum(cS1[:], carries[:].rearrange("p (s k) -> p s k", s=H),
                                     axis=X)
                csq = stp.tile([NCH, C], F32, name="csq")
                nc.scalar.activation(csq[:], carries[:], AF.Square)
                cS2 = stp.tile([NCH, H], F32, name="cS2")
                nc.vector.reduce_sum(cS2[:], csq[:].rearrange("p (s k) -> p s k", s=H),
                                     axis=X)
                cnegm = stp.tile([NCH, H], F32, name="cnegm")
                nc.vector.tensor_scalar(cnegm[:], cS1[:], -1.0 / D, None, op0=AL.mult)
                cmsq = stp.tile([NCH, H], F32, name="cmsq")
                nc.vector.tensor_tensor(cmsq[:], cnegm[:], cnegm[:], op=AL.mult)
                nc.vector.tensor_scalar(cmsq[:], cmsq[:], -1.0, None, op0=AL.mult)
                cv = stp.tile([NCH, H], F32, name="cv")
                nc.vector._custom_dve(AFFINE_THEN_ADD, out=cv[:], in0=cS2[:],
                                      in1=cmsq[:], s0=1.0 / D, s1=EPS)
                cr = _newton_rsqrt(nc, stp, cv[:], NCH, H, "c")
                if f_cgb:
                    nrm32 = stp.tile([NCH, C], F32, name="nrm32")
                    for h in range(H):
                        sl = slice(h * D, (h + 1) * D)
                        nc.vector.tensor_scalar(nrm32[:, sl], carries[:, sl],
                                                cnegm[:, h:h + 1], cr[:, h:h + 1],
                                                op0=AL.add, op1=AL.mult)
                    nc.vector.tensor_tensor(nrm32[:], nrm32[:], cgb_t[:, 0:C],
                                            op=AL.mult)
                    nrm = stp.tile([NCH, C], BF16, name="nrm")
                    nc.vector.tensor_tensor(nrm[:], nrm32[:], cgb_t[:, C:2 * C],
                                            op=AL.add)
                else:
                    def cb(t):
                        ap = t[:]
                        return bass.AP(ap.tensor, ap.offset,
                                       [ap.ap[0], [1, H], [0, D]])
                    nrm = stp.tile([NCH, C], BF16, name="nrm")
                    nc.vector.tensor_tensor(
                        nrm[:].rearrange("p (s k) -> p s k", s=H),
                        carries[:].rearrange("p (s k) -> p s k", s=H),
                        cb(cnegm), op=AL.add)
                    nc.vector.tensor_tensor(
                        nrm[:].rearrange("p (s k) -> p s k", s=H),
                        nrm[:].rearrange("p (s k) -> p s k", s=H),
                        cb(cr), op=AL.mult)
                # normalized carry row j -> partition 127 of gm chunk j; the
                # tri stationary's all-ones row 127 folds it into the cumsum
                nc.gpsimd.dma_start(
                    gmbig[127:128, :].rearrange("p (s k) -> p s k", s=NCH),
                    nrm[:])

            # ============ loop2: cumsum+stats / normalize+MLP, sw-pipelined ==
            ra = stp.tile([128, NCH * H], F32, name="ra")
            negmra = stp.tile([128, NCH * H], F32, name="negmra")
            S1p = stp.tile([128, NCH], F32, name="S1p")
            S2p = stp.tile([128, NCH], F32, name="S2p")
            negmp = stp.tile([128, NCH], F32, name="negmp")
            msqp = stp.tile([128, NCH], F32, name="msqp")
            vp = stp.tile([128, NCH], F32, name="vp")
            rp = stp.tile([128, NCH], F32, name="rp")
            Qs_t = [None] * NCH
            Ys_t = [None] * NCH
            Xr_t = [None] * NCH

            def bcast_d(t, j):
                # [128, 16] stat cols for chunk j broadcast along d=64
                ap = t[:, j * H:(j + 1) * H]
                return bass.AP(ap.tensor, ap.offset, [ap.ap[0], [1, H], [0, D]])

            with tc.tile_pool(name="psqh", bufs=1, space="PSUM") as psqh, \
                 tc.tile_pool(name="psd", bufs=2, space="PSUM") as psd, \
                 tc.tile_pool(name="psp", bufs=1, space="PSUM") as psp, \
                 tc.tile_pool(name="pzt", bufs=1, space="PSUM") as pzt:

                S1a = stp.tile([128, NCH * H], F32, name="S1a")
                S2a = stp.tile([128, NCH * H], F32, name="S2a")

                def emit_tri(j):
                    q = psqh.tile([128, C], F32, name="q", tag="qh")
                    gm = gmbig[:, j * C:(j + 1) * C]
                    nc.tensor.matmul(q[:, 0:512], tri_t[:], gm[:, 0:512],
                                     start=True, stop=True)
                    nc.tensor.matmul(q[:, 512:1024], tri_t[:], gm[:, 512:1024],
                                     start=True, stop=True)
                    return q

                def emit_stats(j, q):
                    qs = bb.tile([128, C], BF16, name="qs", tag="qs", bufs=6)
                    nc.scalar.activation(qs[:], q[:], AF.Copy)
                    qs3 = qs[:].rearrange("p (s k) -> p s k", s=H)
                    nc.vector.reduce_sum(S1a[:, j * H:(j + 1) * H], qs3, axis=X)
                    qsq = bb.tile([128, C], BF16, name="qsq", tag="qsq", bufs=2)
                    nc.gpsimd.tensor_tensor(qsq[:], qs[:], qs[:], op=AL.mult)
                    nc.vector.reduce_sum(S2a[:, j * H:(j + 1) * H],
                                         qsq[:].rearrange("p (s k) -> p s k", s=H),
                                         axis=X)
                    Qs_t[j] = qs

                def emit_cardstats(j0, n):
                    W = n * H
                    cols = slice(j0 * H, j0 * H + W)
                    negma = stp.tile([128, W], F32, name=f"cnegma{n}")
                    nc.vector.tensor_scalar(negma[:], S1a[:, cols], -1.0 / D,
                                            None, op0=AL.mult)
                    msq = stp.tile([128, W], F32, name=f"cms{n}")
                    nc.vector.tensor_tensor(msq[:], negma[:], negma[:], op=AL.mult)
                    nc.vector.tensor_scalar(msq[:], msq[:], -1.0, None, op0=AL.mult)
                    va = stp.tile([128, W], F32, name=f"cva{n}")
                    nc.vector._custom_dve(AFFINE_THEN_ADD, out=va[:], in0=S2a[:, cols],
                                          in1=msq[:], s0=1.0 / D, s1=EPS)
                    rr = _newton_rsqrt(nc, stp, va[:], 128, W, f"a{n}")
                    nc.vector.tensor_tensor(negmra[:, cols], negma[:], rr[:],
                                            op=AL.mult)
                    nc.vector.tensor_copy(ra[:, cols], rr[:])

                def emit_z(j):
                    qs = Qs_t[j]
                    z = bb.tile([128, C], BF16, name="z", tag="z", bufs=3)
                    q3 = qs[:].rearrange("p (s k) -> p s k", s=H)
                    z3 = z[:].rearrange("p (s k) -> p s k", s=H)
                    nc.vector.tensor_tensor(z3, q3, bcast_d(ra, j), op=AL.mult)
                    nc.vector.tensor_tensor(z3, z3, bcast_d(negmra, j), op=AL.add)
                    zts = bb.tile([128, 8 * CS], BF16, name="zts", tag="zts", bufs=2)
                    for half in range(2):
                        zt = pzt.tile([128, 8 * CS // 2], BF16, name=f"zt{half}",
                                      tag=f"zt{half}")
                        for qq in range(4):
                            q2 = half * 4 + qq
                            nc.tensor.matmul(zt[:, qq * CS:(qq + 1) * CS],
                                             z[:, q2 * 128:(q2 + 1) * 128],
                                             ident_t[:], is_transpose=True,
                                             start=(qq == 0), stop=(qq == 3),
                                             skip_group_check=True)
                        nc.vector.tensor_copy(
                            zts[:, half * 512:(half + 1) * 512], zt[:])
                    return zts

                def emit_mlp(j, zts):

                    h1s_par = []
                    for par in (0, 1):
                        h1s = bb.tile([128, 8 * CS], BF16, name="h1s", tag="h1s", bufs=2)
                        for half in range(2):
                            dst = psd.tile([128, 512], F32, name="h1", tag="h1")
                            for qq in range(4):
                                q2 = half * 4 + qq
                                rhs = xb_t[q2][par * 64:par * 64 + 64,
                                               j * CS:(j + 1) * CS]
                                nc.tensor.matmul(dst[:, qq * CS:(qq + 1) * CS],
                                                 w1x_t[par * 64:par * 64 + 64, :],
                                                 rhs,
                                                 start=(qq == 0), stop=False,
                                                 tile_position=(par * 64, 0),
                                                 skip_group_check=True)
                            zsl = zts[par * 64:par * 64 + 64,
                                      half * 512:(half + 1) * 512]
                            nc.tensor.matmul(dst[:],
                                             w1z_t[par * 64:par * 64 + 64, :], zsl,
                                             start=False, stop=not f_h1b,
                                             tile_position=(par * 64, 0),
                                             skip_group_check=True)
                            if f_h1b:
                                nc.tensor.matmul(
                                    dst[:], w1b_t[:],
                                    onesN_t[0:1, half * 512:(half + 1) * 512],
                                    start=False, stop=True,
                                    tile_position=(0, 0), skip_group_check=True)
                            nc.scalar.activation(
                                h1s[:, half * 512:(half + 1) * 512], dst[:],
                                AF.Gelu)
                        h1s_par.append(h1s)
                    h1se, h1so = h1s_par

                    hop = psqh.tile([128, 8 * CS], F32, name="hop", tag="qh")
                    hops = bb.tile([128, 8 * CS], BF16, name="hops", tag="hops", bufs=2)
                    for bank in range(2):
                        cs_ = slice(bank * 512, (bank + 1) * 512)
                        for par, h1s in ((0, h1se), (1, h1so)):
                            pr = slice(par * 64, par * 64 + 64)
                            nc.tensor.matmul(hop[pr, cs_], w2_t[:], h1s[:, cs_],
                                             start=True, stop=not f_h2b,
                                             tile_position=(0, 64 * par),
                                             skip_group_check=True)
                        if not f_h2b:
                            # gpsimd cannot read PSUM; split across DVE/Act
                            if bank == 0:
                                nc.vector.tensor_copy(hops[:, cs_], hop[:, cs_])
                            else:
                                nc.scalar.activation(hops[:, cs_], hop[:, cs_],
                                                     AF.Copy)
                    if f_h2b:
                        for par in (0, 1):
                            pr = slice(par * 64, par * 64 + 64)
                            nc.tensor.matmul(hop[pr, 0:512], w2b_t[:],
                                             onesN_t[0:1, 0:512], start=False,
                                             stop=False, tile_position=(0, 64 * par))
                            nc.tensor.matmul(hop[pr, 512:1024], w2b_t[:],
                                             onesN_t[0:1, 512:1024], start=False,
                                             stop=(par == 1), tile_position=(0, 64 * par))
                    if f_h2b:
                        nc.vector.tensor_copy(hops[:], hop[:])

                    pj = psp.tile([128, C], F32, name="pj")
                    for q2 in range(8):
                        st = hops[:, q2 * CS:(q2 + 1) * CS]
                        last = (q2 == 7) and not f_projb
                        nc.tensor.matmul(pj[:, 0:512], st, wproj_t[q2][:, 0:512],
                                         start=(q2 == 0), stop=last)
                        nc.tensor.matmul(pj[:, 512:1024], st, wproj_t[q2][:, 512:1024],
                                         start=(q2 == 0), stop=last)
                    if f_projb:
                        nc.tensor.matmul(pj[:, 0:512], ones1_t[0:1, :], wprojb_t[0:1, 0:512],
                                         start=False, stop=True)
                        nc.tensor.matmul(pj[:, 512:1024], ones1_t[0:1, :],
                                         wprojb_t[0:1, 512:1024], start=False, stop=True)
                    ys = cp.tile([128, C], BF16, name="ys", tag=f"wsl{j % 6}")
                    nc.scalar.activation(ys[:], pj[:], AF.Copy,
                                         accum_out=S1p[:, j:j + 1])
                    Ys_t[j] = ys
                    sqd = cp.tile([128, C], BF16, name="sqd", tag="wsl6")
                    nc.scalar.activation(sqd[:], ys[:], AF.Square,
                                         accum_out=S2p[:, j:j + 1])
                    xr = bb.tile([128, C], BF16, name="xr", tag="xr", bufs=5)
                    nc.gpsimd.dma_start(xr[:], xres[j * CS:(j + 1) * CS, :])
                    Xr_t[j] = xr

                def emit_tail(j0, n):
                    cols = slice(j0, j0 + n)
                    nc.vector.tensor_scalar(negmp[:, cols], S1p[:, cols],
                                            -1.0 / C, None, op0=AL.mult)
                    nc.vector.tensor_tensor(msqp[:, cols], negmp[:, cols],
                                            negmp[:, cols], op=AL.mult)
                    nc.vector.tensor_scalar(msqp[:, cols], msqp[:, cols],
                                            -1.0, None, op0=AL.mult)
                    nc.vector._custom_dve(AFFINE_THEN_ADD, out=vp[:, cols],
                                          in0=S2p[:, cols],
                                          in1=msqp[:, cols], s0=1.0 / C, s1=EPS)
                    rg = _newton_rsqrt(nc, stp, vp[:, cols], 128, n, f"p{n}")
                    nc.vector.tensor_copy(rp[:, cols], rg[:])
                    for k in range(n):
                        j = j0 + k
                        ys, xr = Ys_t[j], Xr_t[j]
                        ost = strm.tile([128, C], BF16, name="ost", tag="ost")
                        if f_lng:
                            t1 = strm.tile([128, C], F32, name="lnt", tag="lnt")
                            nc.vector.tensor_scalar(t1[:], ys[:], negmp[:, j:j + 1],
                                                    rp[:, j:j + 1],
                                                    op0=AL.add, op1=AL.mult)
                            nc.vector.tensor_tensor(t1[:], t1[:], lngb_t[:], op=AL.mult)
                            nc.vector.tensor_tensor(ost[:], t1[:], xr[:], op=AL.add)
                        else:
                            tmp = strm.tile([128, C], BF16, name="lnb", tag="lnb")
                            nc.gpsimd.tensor_scalar(tmp[:], ys[:], negmp[:, j:j + 1],
                                                    rp[:, j:j + 1],
                                                    op0=AL.add, op1=AL.mult)
                            nc.vector.tensor_tensor(ost[:], tmp[:], xr[:], op=AL.add)
                        nc.gpsimd.dma_start(out[j * CS:(j + 1) * CS, :], ost[:])

                sizes = [2] * 8
                starts = list(range(0, NCH, 2))
                for gi in range(len(sizes)):
                    j0, n = starts[gi], sizes[gi]
                    j0p, np_ = (starts[gi - 1], sizes[gi - 1]) if gi else (0, 0)
                    for k in range(max(n, np_)):
                        q = emit_tri(j0 + k) if k < n else None
                        zts = emit_z(j0p + k) if k < np_ else None
                        if q is not None:
                            emit_stats(j0 + k, q)
                        if zts is not None:
                            emit_mlp(j0p + k, zts)
                    emit_cardstats(j0, n)
                    if gi:
                        emit_tail(j0p, np_)
                j0p, np_ = starts[-1], sizes[-1]
                for k in range(np_):
                    emit_mlp(j0p + k, emit_z(j0p + k))
                emit_tail(j0p, np_)

    nc.compile()
    return nc


_CACHE = {}
_LAST_RESULTS = [None]


def _to_bf(a):
    return np.ascontiguousarray(np.asarray(a, dtype=np.float32).astype(BFNP))


def _to_f8(a):
    return np.asarray(a, dtype=np.float32).astype(F8NP)


def prepare(x, mark_W, mark_b, gate_W, gate_b, carry_g, carry_b,
            card_g, card_b, ho1_W, ho1_b, ho2_W, ho2_b,
            proj_W, proj_b, ln_g, ln_b):
    x = np.asarray(x, dtype=np.float32)
    mark_W = np.asarray(mark_W, dtype=np.float32)
    mark_b = np.asarray(mark_b, dtype=np.float32)
    gate_W = np.asarray(gate_W, dtype=np.float32)
    gate_b = np.asarray(gate_b, dtype=np.float32)
    carry_g = np.asarray(carry_g, dtype=np.float32)
    carry_b = np.asarray(carry_b, dtype=np.float32)
    card_g = np.asarray(card_g, dtype=np.float32)
    card_b = np.asarray(card_b, dtype=np.float32)
    ho1_W = np.asarray(ho1_W, dtype=np.float32)
    ho1_b = np.asarray(ho1_b, dtype=np.float32)
    ho2_W = np.asarray(ho2_W, dtype=np.float32)
    ho2_b = np.asarray(ho2_b, dtype=np.float32)
    proj_W = np.asarray(proj_W, dtype=np.float32)
    proj_b = np.asarray(proj_b, dtype=np.float32)
    ln_g = np.asarray(ln_g, dtype=np.float32)
    ln_b = np.asarray(ln_b, dtype=np.float32)

    flags = (
        bool(np.any(mark_b) or np.any(gate_b)),
        bool(np.any(proj_b)),
        bool(np.any(ho1_b) or np.any(card_b)),
        bool(np.any(ho2_b)),
        bool(np.any(ln_g != 1.0)),
        bool(np.any(carry_g != 1.0) or np.any(carry_b)),
    )
    # ---- host-side fold + shard prep ----
    # card LN gain folds into the cards half of ho1_W; card bias into ho1_b.
    w1 = ho1_W.copy()
    w1[D:2 * D, :] = w1[D:2 * D, :] * card_g[:, None]
    b1 = ho1_b + card_b @ ho1_W[D:2 * D, :]
    def pack8(a):  # [1024, X] -> [128, 8*X] c-block-major
        return np.ascontiguousarray(
            a.reshape(8, 128, a.shape[1]).transpose(1, 0, 2).reshape(
                128, 8 * a.shape[1]))

    wmg = np.concatenate([mark_W, gate_W], axis=1)       # [C, 2C]
    wmg8_np = pack8(np.concatenate(
        [_to_f8(16.0 * wmg), _to_f8(wmg)], axis=1).view(np.uint8))
    wproj_np = pack8(_to_bf(proj_W))
    w1x_np = _to_bf(np.vstack([w1[0:D, :], w1[0:D, :]]))
    w1z_np = _to_bf(np.vstack([w1[D:2 * D, :], w1[D:2 * D, :]]))
    w2_np = _to_bf(ho2_W)
    tri_np = np.triu(np.ones((CS, CS), np.float32), 1)
    tri_np[CS - 1, :] = 1.0                              # carry row
    tri_np = _to_bf(tri_np)
    tri16_np = _to_bf(np.triu(np.ones((NCH, NCH), np.float32), 1))
    ones16_np = _to_bf(np.ones((NCH, 1), np.float32))
    ones1_np = _to_bf(np.ones((CS, CS), np.float32))
    ejs_np = np.zeros((CS, NCH * NCH), np.float32)
    for j in range(NCH):
        ejs_np[:, j * NCH + j] = 1.0
    ejs_np = _to_bf(ejs_np)
    ident_np = _to_bf(np.eye(CS, dtype=np.float32))

    common = dict(wmg8=wmg8_np, wproj=wproj_np, w1x=w1x_np, w1z=w1z_np, w2=w2_np,
                  tri=tri_np, tri16=tri16_np, ones16=ones16_np, ones1=ones1_np,
                  ejs=ejs_np, ident=ident_np)
    if flags[0]:
        common["wmgb"] = _to_bf(16.0 * np.concatenate([mark_b, gate_b])[None, :])
    if flags[1]:
        common["wprojb"] = _to_bf(proj_b[None, :])
    if flags[2]:
        common["w1b"] = _to_bf(b1[None, :])
    if flags[3]:
        common["w2b"] = _to_bf(ho2_b[None, :])
    if flags[2] or flags[3]:
        common["onesN"] = _to_bf(np.ones((1, 8 * CS), np.float32))
    if flags[4]:
        common["lngb"] = np.ascontiguousarray(
            np.broadcast_to(ln_g[None, :], (128, C)), dtype=np.float32)
    if flags[5]:
        cg = np.broadcast_to(np.tile(carry_g, H)[None, :], (NCH, C))
        cb = np.broadcast_to(np.tile(carry_b, H)[None, :], (NCH, C))
        common["cgb"] = np.ascontiguousarray(
            np.concatenate([cg, cb], axis=1), dtype=np.float32)

    in_maps = []
    for core in range(NCORES):
        b, g = core // 2, core % 2
        rows = slice(g * TL, (g + 1) * TL)
        m = dict(common)
        xt = np.ascontiguousarray(x[b, rows, :].T)       # [C, TL] f32
        hi = _to_f8(xt)
        lo = _to_f8((xt - hi.astype(np.float32)) * 16.0)
        m["xhl"] = pack8(np.concatenate([hi, lo], axis=1).view(np.uint8))
        m["xT"] = pack8(np.ascontiguousarray(xt.astype(BFNP)))
        m["xres"] = np.ascontiguousarray(
            (x[b, rows, :] + ln_b[None, :]).astype(BFNP))
        m["tsel"] = np.full((1, 1), float(g), np.float32)
        in_maps.append(m)
    return flags, in_maps


def assemble(results):
    out = np.empty((B, T, C), np.float32)
    for core in range(NCORES):
        b, g = core // 2, core % 2
        out[b, g * TL:(g + 1) * TL, :] = np.asarray(results[core]["out"]).astype(np.float32)
    return out


def kernel(**inputs):
    flags, in_maps = prepare(**inputs)
    if flags not in _CACHE:
        _CACHE[flags] = build_nc(flags)
    nc = _CACHE[flags]
    res = run_bass_kernel_spmd(nc, in_maps, core_ids=list(range(NCORES)))
    _LAST_RESULTS[0] = res
    return assemble(res.results)
